# revision 26
# baseline (speedup 1.0000x reference)
"""Trainium2 Bass kernel for a Mamba block (LayerNorm -> in_proj -> causal
depthwise conv1d + SiLU -> selective scan (SSM) -> gate -> out_proj).

Full inputs (B=8, L=2048, d_model=128) are sharded batch-parallel across the
8 NeuronCores (one batch element per core, no collectives). The second
reference output, `residual`, equals the input `x` and is returned host-side.

Per-core pipeline (channel-on-partition, time-on-free layout, fp16 compute):
  - LN stats on VectorE (bn_stats/bn_aggr); rstd via exp(-0.5*ln(var+eps))
    so all Act transcendentals stay in one table set; LN affine folded into
    in_proj weights on host.
  - TensorE transposes to (d, t). The causal depthwise conv is FUSED into
    the in_proj matmul: 4 tap-scaled copies of the in_proj weight accumulate
    over shifted xnT windows in PSUM (zero-pad columns on the left); SiLU +
    bias ride the PSUM evacuation on ScalarE.  The z half gets Silu on evac.
  - x_proj / dt_proj matmuls; softplus via Exp+Ln(1+x) on ScalarE.
  - selective scan, n-outer loop: per state n, B/C rows are broadcast via
    the SP HWDGE queue (~0.6us each; issuing from other engine queues
    serializes against their compute); dA = exp(delta*A[:,n]) on ScalarE;
    dBx = u*Bb, the two scans, and hc = hst*Cb ALL on DVE — the dBx/hc
    multiplies as single both-halves [128,2,2048] ops with a stride-0
    broadcast middle dim, which pipeline into DVE's stream nearly free,
    while cross-engine handoffs (GpSimd/split variants) cost 1.5-2.4x
    per-iteration (HW-measured via micro.py loop_* variants);
    y accumulates over n as identity matmuls in PSUM (8 banks).
  - yd = xc*D + y_psum; gate on GpSimd; out_proj lands in (t, d_model).
"""
import os
import numpy as np

D_MODEL, D_INNER, D_STATE, D_CONV, DT_RANK = 128, 256, 16, 4, 8
L = 2048
N_CORES = 8
NT = L // 128          # 16 t-tiles of 128
NC4 = L // 512         # 4 t-chunks of 512

_cache = {}


def _build(reps=1, legalize=True, hc_eng="vector", dbx_eng="vector", skip=(),
           silu_mode="act", crossrep=False):
    import concourse.bass as bass
    import concourse.tile as tile
    from concourse import mybir
    from concourse import masks

    f32 = mybir.dt.float32
    f16 = mybir.dt.float16
    ts = bass.ts
    Alu = mybir.AluOpType
    Act = mybir.ActivationFunctionType

    nc = bass.Bass()

    # ---- DRAM I/O (per core) ----
    x_d = nc.dram_tensor("x", [L, D_MODEL], f32, kind="ExternalInput")
    w1conv_d = nc.dram_tensor("w1conv", [128, 2, D_CONV, 128], f16, kind="ExternalInput")
    w1z_d = nc.dram_tensor("w1z", [128, 2, 128], f16, kind="ExternalInput")
    biasc_d = nc.dram_tensor("biasc", [128, 2], f32, kind="ExternalInput")
    biasz_d = nc.dram_tensor("biasz", [128, 2], f32, kind="ExternalInput")
    corr3_d = nc.dram_tensor("corr3", [128, 2, 3], f32, kind="ExternalInput")
    xpt_d = nc.dram_tensor("xpt", [128, 2, 96], f16, kind="ExternalInput")
    dtpt_d = nc.dram_tensor("dtpt", [DT_RANK, D_INNER], f16, kind="ExternalInput")
    dtb_d = nc.dram_tensor("dtb", [128, 2], f32, kind="ExternalInput")
    A_d = nc.dram_tensor("A", [128, 2, D_STATE], f32, kind="ExternalInput")
    Dp_d = nc.dram_tensor("Dp", [128, 2], f32, kind="ExternalInput")
    w2t_d = nc.dram_tensor("w2t", [128, 2, D_MODEL], f16, kind="ExternalInput")
    out_d = nc.dram_tensor("out", [L, D_MODEL], f32, kind="ExternalOutput")
    bc_d = nc.dram_tensor("bc_scratch", [2, D_STATE, L], f16, kind="Internal")

    eng = {"vector": nc.vector, "gpsimd": nc.gpsimd, "scalar": nc.scalar}
    hce = eng.get(hc_eng, nc.vector)
    dbe = eng.get(dbx_eng, nc.vector)

    with tile.TileContext(nc) as tc:
        with (
            tc.tile_pool(name="singles", bufs=1) as singles,
            tc.tile_pool(name="big", bufs=1) as big,
            tc.tile_pool(name="ln", bufs=4) as lnp,
            tc.tile_pool(name="scan", bufs=(2 if crossrep else 3)) as scanp,
            tc.tile_pool(name="bcast", bufs=3) as bcastp,
            tc.tile_pool(name="pp", bufs=1, space="PSUM") as pp,
        ):
            # ---- load weights ----
            w1conv = singles.tile([128, 2, D_CONV, 128], f16)
            nc.sync.dma_start(w1conv, w1conv_d[:])
            w1z = singles.tile([128, 2, 128], f16)
            nc.sync.dma_start(w1z, w1z_d[:])
            biasc = singles.tile([128, 2], f32)
            nc.sync.dma_start(biasc, biasc_d[:])
            biasz = singles.tile([128, 2], f32)
            nc.sync.dma_start(biasz, biasz_d[:])
            corr3 = singles.tile([128, 2, 3], f32)
            nc.sync.dma_start(corr3, corr3_d[:])
            xpt = singles.tile([128, 2, 96], f16)
            nc.sync.dma_start(xpt, xpt_d[:])
            dtpt = singles.tile([DT_RANK, D_INNER], f16)
            nc.sync.dma_start(dtpt, dtpt_d[:])
            dtb = singles.tile([128, 2], f32)
            nc.sync.dma_start(dtb, dtb_d[:])
            A_sb = singles.tile([128, 2, D_STATE], f32)
            nc.sync.dma_start(A_sb, A_d[:])
            Dp = singles.tile([128, 2], f32)
            nc.sync.dma_start(Dp, Dp_d[:])
            w2t = singles.tile([128, 2, D_MODEL], f16)
            nc.sync.dma_start(w2t, w2t_d[:])
            ident = singles.tile([128, 128], f16)
            masks.make_identity(nc, ident[:])
            eps = singles.tile([128, 1], f32)
            nc.vector.memset(eps, 1e-5)
            if silu_mode == "tanh":
                # silu(x) = x*0.5*(1+tanh(x/2)): halved biases for the
                # scale=0.5 activation reads
                biasc_h = singles.tile([128, 2], f32)
                nc.scalar.mul(biasc_h, biasc, 0.5)
                biasz_h = singles.tile([128, 2], f32)
                nc.scalar.mul(biasz_h, biasz, 0.5)

            bk = [f"bk{i}" for i in range(8)]
            xb = 2 if crossrep else 1

            for _rep in range(reps):
                # ---- load x: (2048, 128) -> (128 part, 16, 128) ----
                x_sb = big.tile([128, NT, D_MODEL], f32,
                                tag=("xin" if crossrep else "xio"), bufs=xb)
                nc.sync.dma_start(x_sb, x_d.rearrange("(i p) d -> p i d", p=128))

                # ---- LayerNorm (stats per t-row; t on partitions) ----
                # batched: all 16 tiles' stats first (DVE pipelines), then the
                # mean/var -> rstd/nmr math as a few [128,16]-wide ops
                xn16 = big.tile([128, NT, D_MODEL], f16)
                mv = lnp.tile([128, NT, 2], f32, tag="mv")
                for i in range(NT):
                    st = lnp.tile([128, 6], f32, tag=f"st{i % 4}")
                    nc.vector.bn_stats(st, x_sb[:, i, :])
                    nc.vector.bn_aggr(mv[:, i, :], st)
                # rstd = exp(-0.5*ln(var+eps)) — stays in the exp/ln
                # activation-table set (no sqrt table load)
                lv = lnp.tile([128, NT], f32, tag="lv")
                nc.scalar.activation(lv, mv[:, :, 1], Act.Ln, bias=eps[:])
                rstd = lnp.tile([128, NT], f32, tag="rstd")
                nc.scalar.activation(rstd, lv, Act.Exp, scale=-0.5)
                nmr = lnp.tile([128, NT], f32, tag="nmr")
                # nmr = -(mu * rstd)
                nc.vector.tensor_tensor(nmr, mv[:, :, 0], rstd, op=Alu.mult)
                nc.vector.tensor_scalar(nmr, nmr, -1.0, 0.0,
                                        op0=Alu.mult, op1=Alu.add)
                for i in range(NT):
                    nc.scalar.activation(xn16[:, i, :], x_sb[:, i, :], Act.Identity,
                                         bias=nmr[:, i:i + 1], scale=rstd[:, i:i + 1])

                # ---- transpose xn -> (d_model, 3+t) with 3 left pad cols ----
                xnT = big.tile([128, 3 + L], f16)
                nc.vector.memset(xnT[:, 0:3], 0.0)
                for i in range(NT):
                    pt = pp.tile([128, 128], f16, tag=bk[i % 6])
                    nc.tensor.transpose(pt, xn16[:, i, :], ident)
                    if i % 2 == 0:
                        nc.vector.tensor_copy(xnT[:, 3 + i * 128:3 + (i + 1) * 128], pt)
                    else:
                        nc.scalar.copy(xnT[:, 3 + i * 128:3 + (i + 1) * 128], pt)

                # ---- in_proj + fused causal conv (xc half), silu on evac ----
                xc2 = big.tile([128, 2, L], f16, name="xc2")
                sz2 = big.tile([128, 2, L], f16, name="sz2")

                def silu_evac(dst, pz, bias, bias_h, hh):
                    if silu_mode == "tanh":
                        # stays in the exp/ln/tanh act-table set (no
                        # swish-set reload): x*0.5*(1+tanh(x/2))
                        w = lnp.tile([128, 512], f16, tag="siw", bufs=3)
                        nc.scalar.activation(w, pz, Act.Identity, scale=0.5,
                                             bias=bias_h[:, hh:hh + 1])
                        t = lnp.tile([128, 512], f16, tag="sit", bufs=3)
                        nc.scalar.activation(t, pz, Act.Tanh, scale=0.5,
                                             bias=bias_h[:, hh:hh + 1])
                        nc.gpsimd.scalar_tensor_tensor(dst, t, 1.0, w,
                                                       op0=Alu.add, op1=Alu.mult)
                    else:
                        nc.scalar.activation(dst, pz, Act.Silu,
                                             bias=bias[:, hh:hh + 1])

                for h in range(2):
                    for tn in range(NC4):
                        pz = pp.tile([128, 512], f32, tag=bk[(h * 4 + tn) % 8])
                        for k in range(D_CONV):
                            nc.tensor.matmul(pz, w1conv[:, h, k, :],
                                             xnT[:, tn * 512 + k:tn * 512 + k + 512],
                                             start=(k == 0), stop=(k == D_CONV - 1))
                        if tn == 0:
                            # left-pad boundary: remove phantom-bias taps
                            nc.vector.tensor_tensor(pz[:, 0:3], pz[:, 0:3],
                                                    corr3[:, h, :], op=Alu.subtract)
                        silu_evac(xc2[:, h, ts(tn, 512)], pz, biasc,
                                  biasc_h if silu_mode == "tanh" else biasc, h)
                # ---- in_proj z half: sz = silu(z) on evac ----
                for g in range(2):
                    for tn in range(NC4):
                        pz = pp.tile([128, 512], f32, tag=bk[(g * 4 + tn) % 8])
                        nc.tensor.matmul(pz, w1z[:, g, :],
                                         xnT[:, 3 + tn * 512:3 + (tn + 1) * 512],
                                         start=True, stop=True)
                        silu_evac(sz2[:, g, ts(tn, 512)], pz, biasz,
                                  biasz_h if silu_mode == "tanh" else biasz, g)

                # ---- x_proj: dbc = x_proj_w @ xc  (96 rows: dt/B/C) ----
                dbc_sb = big.tile([96, L], f16, name="dbc")
                for tn in range(NC4):
                    pd = pp.tile([96, 512], f32, tag=bk[tn % 8])
                    nc.tensor.matmul(pd, xpt[:, 0, :], xc2[:, 0, ts(tn, 512)],
                                     start=True, stop=False)
                    nc.tensor.matmul(pd, xpt[:, 1, :], xc2[:, 1, ts(tn, 512)],
                                     start=False, stop=True)
                    nc.vector.tensor_copy(dbc_sb[:, ts(tn, 512)], pd)
                nc.sync.dma_start(bc_d[0], dbc_sb[32:32 + D_STATE, :])
                nc.sync.dma_start(bc_d[1], dbc_sb[64:64 + D_STATE, :])

                # ---- delta = softplus(dt_proj_w @ dt + b)  (d on partitions) ----
                delta2 = big.tile([128, 2, L], f16, name="delta2", bufs=xb)
                for h in range(2):
                    for tn in range(NC4):
                        pdl = pp.tile([128, 512], f32, tag=bk[4 + (h * 4 + tn) % 4])
                        nc.tensor.matmul(pdl, dtpt[:, ts(h, 128)],
                                         dbc_sb[0:DT_RANK, ts(tn, 512)],
                                         start=True, stop=True)
                        edl = lnp.tile([128, 512], f32, tag="edl", bufs=2)
                        nc.scalar.activation(edl, pdl, Act.Exp, bias=dtb[:, h:h + 1])
                        nc.scalar.activation(delta2[:, h, ts(tn, 512)], edl,
                                             Act.Ln, bias=1.0)

                # ---- u = delta * xc ----
                u2 = big.tile([128, 2, L], f16, name="u2", bufs=xb)
                nc.vector.tensor_tensor(u2, delta2, xc2, op=Alu.mult)

                # ---- selective scan, n-outer; y accumulates in PSUM ----
                py = [pp.tile([128, 512], f32, tag=bk[j], name=f"py{j}")
                      for j in range(8)]
                if "scanloop" in skip:
                    for j in range(8):
                        nc.tensor.matmul(py[j], ident, u2[:, j // 4, ts(j % 4, 512)],
                                         start=True, stop=True)
                for n in range(D_STATE if "scanloop" not in skip else 0):
                    if "bcast" in skip:
                        Bb = delta2[:, 0, :]
                        Cb = delta2[:, 1, :]
                    else:
                        # both on the SP HWDGE queue — issuing Cb from the
                        # scalar engine serializes against the dA exps
                        Bb = bcastp.tile([128, L], f16, tag="Bb")
                        nc.sync.dma_start(Bb, bc_d[0, n:n + 1, :].broadcast_to([128, L]))
                        Cb = bcastp.tile([128, L], f16, tag="Cb")
                        nc.sync.dma_start(Cb, bc_d[1, n:n + 1, :].broadcast_to([128, L]))

                    if "dA" in skip:
                        dA2 = u2
                    else:
                        dA2 = scanp.tile([128, 2, L], f16, tag="dA2")
                        for h in range(2):
                            nc.scalar.activation(dA2[:, h, :], delta2[:, h, :], Act.Exp,
                                                 scale=A_sb[:, h, n:n + 1])
                    if "dbx" in skip:
                        dBx2 = u2
                    else:
                        dBx2 = scanp.tile([128, 2, L], f16, tag="dBx2")
                        if dbx_eng == "split":
                            for h in range(2):
                                e = nc.vector if h == 0 else nc.gpsimd
                                e.tensor_tensor(dBx2[:, h, :], u2[:, h, :], Bb,
                                                op=Alu.mult)
                        elif dbx_eng in ("vector2", "gpsimd2"):
                            e = nc.vector if dbx_eng == "vector2" else nc.gpsimd
                            for h in range(2):
                                e.tensor_tensor(dBx2[:, h, :], u2[:, h, :], Bb,
                                                op=Alu.mult)
                        else:
                            dbe.tensor_tensor(dBx2, u2,
                                              Bb.unsqueeze(1).broadcast_to([128, 2, L]),
                                              op=Alu.mult)
                    if "scan" in skip:
                        hst2 = dBx2
                    else:
                        hst2 = scanp.tile([128, 2, L], f16, tag="hst2")
                        for h in range(2):
                            nc.vector.tensor_tensor_scan(hst2[:, h, :], dA2[:, h, :],
                                                         dBx2[:, h, :], 0.0,
                                                         op0=Alu.mult, op1=Alu.add)
                    if "hc" in skip:
                        hc2 = hst2
                    else:
                        hc2 = scanp.tile([128, 2, L], f16, tag="hc2")
                        if hc_eng == "split":
                            for h in range(2):
                                e = nc.gpsimd if h == 0 else nc.vector
                                e.tensor_tensor(hc2[:, h, :], hst2[:, h, :], Cb,
                                                op=Alu.mult)
                        elif hc_eng in ("vector2", "gpsimd2"):
                            e = nc.vector if hc_eng == "vector2" else nc.gpsimd
                            for h in range(2):
                                e.tensor_tensor(hc2[:, h, :], hst2[:, h, :], Cb,
                                                op=Alu.mult)
                        else:
                            hce.tensor_tensor(hc2, hst2,
                                              Cb.unsqueeze(1).broadcast_to([128, 2, L]),
                                              op=Alu.mult)
                    if "accum" in skip:
                        if n == 0:
                            for j in range(8):
                                nc.tensor.matmul(py[j], ident,
                                                 hc2[:, j // 4, ts(j % 4, 512)],
                                                 start=True, stop=True)
                    else:
                        for h in range(2):
                            for c in range(NC4):
                                nc.tensor.matmul(py[h * 4 + c], ident,
                                                 hc2[:, h, ts(c, 512)],
                                                 start=(n == 0), stop=(n == D_STATE - 1))

                # ---- yd = (xc * D) + y_psum;  gate ----
                yd2 = big.tile([128, 2, L], f16, name="yd2")
                for h in range(2):
                    for c in range(NC4):
                        nc.vector.scalar_tensor_tensor(
                            yd2[:, h, ts(c, 512)], xc2[:, h, ts(c, 512)],
                            Dp[:, h:h + 1], py[h * 4 + c], op0=Alu.mult, op1=Alu.add)
                yg2 = big.tile([128, 2, L], f16, name="yg2")
                nc.vector.tensor_tensor(yg2, yd2, sz2, op=Alu.mult)

                # ---- out_proj: out[t, dm] = sum_c yg[c, t] * w2t[c, dm] ----
                out_sb = big.tile([128, NT, D_MODEL], f32, tag="xio")
                for i in range(NT):
                    po = pp.tile([128, D_MODEL], f32, tag=bk[i % 4])
                    nc.tensor.matmul(po, yg2[:, 0, ts(i, 128)], w2t[:, 0, :],
                                     start=True, stop=False)
                    nc.tensor.matmul(po, yg2[:, 1, ts(i, 128)], w2t[:, 1, :],
                                     start=False, stop=True)
                    if i % 2 == 0:
                        nc.vector.tensor_copy(out_sb[:, i, :], po)
                    else:
                        nc.scalar.copy(out_sb[:, i, :], po)

                nc.sync.dma_start(out_d.rearrange("(i p) d -> p i d", p=128), out_sb)

    if legalize:
        _legalize_waits(nc)
    return nc


def _legalize_waits(nc):
    """This container's walrus codegen rejects instructions carrying more
    than one sync wait. Hoist extra waits onto preceding wait-only
    InstEventSemaphore instructions on the same engine (sequencers execute
    them in order, so the semantics are identical)."""
    from concourse import mybir

    fixid = [0]
    for fn in nc.m.functions:
        for blk in fn.blocks:
            out = []
            changed = False
            for ins in blk.instructions:
                si = getattr(ins, "sync_info", None)
                waits = list(si.on_wait) if si is not None and si.on_wait else []
                if len(waits) > 1:
                    for w in waits[:-1]:
                        fixid[0] += 1
                        nop = mybir.InstEventSemaphore(
                            name=f"I-waitfix-{fixid[0]}", ins=[], outs=[],
                            sync_info=mybir.SyncInfo(on_wait=[w], on_update=[]))
                        nop.engine = ins.engine
                        out.append(nop)
                    ins.sync_info = mybir.SyncInfo(
                        on_wait=[waits[-1]], on_update=list(si.on_update))
                    changed = True
                out.append(ins)
            if changed:
                blk.instructions = out


def _prep_inputs(x, norm_w, norm_b, in_proj_w, conv_w, conv_b, x_proj_w,
                 dt_proj_w, dt_proj_b, A_log, D, out_proj_w):
    """Host-side weight prep; returns per-core input maps."""
    f32 = np.float32
    f16 = np.float16
    W1eff = (in_proj_w.astype(f32) * norm_w.astype(f32)[None, :])  # (512, 128)
    b1 = (in_proj_w.astype(f32) @ norm_b.astype(f32))              # (512,)
    cw = conv_w.astype(f32)                                        # (256, 4)

    # xc half: 4 tap-scaled stationaries  w1conv[d, h, k, j] = W1eff[h*128+j, d]*cw[h*128+j, k]
    Wt = W1eff[:256].T                                             # (128d, 256c)
    w1conv = np.empty((128, 2, D_CONV, 128), f32)
    for h in range(2):
        for k in range(D_CONV):
            w1conv[:, h, k, :] = Wt[:, h * 128:(h + 1) * 128] * cw[h * 128:(h + 1) * 128, k][None, :]
    w1conv = w1conv.astype(f16)
    w1z = np.ascontiguousarray(
        W1eff[256:].T.reshape(128, 2, 128)).astype(f16)
    # conv bias folded with in_proj bias (all 4 taps of b1)
    biasc = (conv_b.astype(f32) + b1[:256] * cw.sum(1)).reshape(2, 128).T
    biasc = np.ascontiguousarray(biasc)
    biasz = np.ascontiguousarray(b1[256:].reshape(2, 128).T)
    # boundary correction: output col j<3 should not include b1 for taps k<3-j
    corr3 = np.empty((128, 2, 3), f32)
    for h in range(2):
        for j in range(3):
            corr3[:, h, j] = b1[h * 128:(h + 1) * 128] * cw[h * 128:(h + 1) * 128, :3 - j].sum(1)
    corr3 = np.ascontiguousarray(corr3)

    xpw_pad = np.zeros((96, 256), f32)
    xpw_pad[0:8] = x_proj_w[0:8]
    xpw_pad[32:48] = x_proj_w[8:24]
    xpw_pad[64:80] = x_proj_w[24:40]
    xpt = np.ascontiguousarray(
        xpw_pad.T.reshape(2, 128, 96).transpose(1, 0, 2)).astype(f16)
    dtpt = np.ascontiguousarray(dt_proj_w.astype(f32).T).astype(f16)  # (8, 256)
    dtb = np.ascontiguousarray(dt_proj_b.astype(f32).reshape(2, 128).T)
    A = (-np.exp(A_log.astype(f32)))
    A = np.ascontiguousarray(A.reshape(2, 128, D_STATE).transpose(1, 0, 2))
    Dp = np.ascontiguousarray(D.astype(f32).reshape(2, 128).T)
    w2t = np.ascontiguousarray(
        out_proj_w.astype(f32).T.reshape(2, 128, D_MODEL).transpose(1, 0, 2)).astype(f16)

    shared = dict(w1conv=w1conv, w1z=w1z, biasc=biasc, biasz=biasz, corr3=corr3,
                  xpt=xpt, dtpt=dtpt, dtb=dtb, A=A, Dp=Dp, w2t=w2t)
    in_maps = []
    for b in range(N_CORES):
        m = dict(shared)
        m["x"] = np.ascontiguousarray(x[b].astype(f32))
        in_maps.append(m)
    return in_maps


def kernel(**inputs):
    from concourse.bass_utils import run_bass_kernel_spmd

    if "nc" not in _cache:
        _cache["nc"] = _build()
    nc = _cache["nc"]

    x = np.asarray(inputs["x"])
    in_maps = _prep_inputs(**{k: np.asarray(v) for k, v in inputs.items()})
    res = run_bass_kernel_spmd(nc, in_maps, list(range(N_CORES)),
                               trace=bool(int(os.environ.get("KTRACE", "0"))))
    _cache["last_results"] = res
    out = np.stack([res.results[b]["out"] for b in range(N_CORES)]).astype(np.float32)
    residual = x.astype(np.float32).copy()
    return out, residual


# revision 32
# speedup vs baseline: 1.1521x; 1.1521x over previous
"""Trainium2 Bass kernel for a Mamba block (LayerNorm -> in_proj -> causal
depthwise conv1d + SiLU -> selective scan (SSM) -> gate -> out_proj).

Full inputs (B=8, L=2048, d_model=128) are sharded batch-parallel across the
8 NeuronCores (one batch element per core, no collectives). The second
reference output, `residual`, equals the input `x` and is returned host-side.

Per-core pipeline (channel-on-partition, time-on-free layout, fp16 compute):
  - LN stats on VectorE (bn_stats/bn_aggr); rstd via exp(-0.5*ln(var+eps))
    so all Act transcendentals stay in one table set; LN affine folded into
    in_proj weights on host.
  - TensorE transposes to (d, t). The causal depthwise conv is FUSED into
    the in_proj matmul: 4 tap-scaled copies of the in_proj weight accumulate
    over shifted xnT windows in PSUM (zero-pad columns on the left); SiLU +
    bias ride the PSUM evacuation on ScalarE.  The z half gets Silu on evac.
  - x_proj / dt_proj matmuls; softplus via Exp+Ln(1+x) on ScalarE.
  - selective scan, n-outer loop: per state n, B/C rows are broadcast via
    the SP HWDGE queue (~0.6us each; issuing from other engine queues
    serializes against their compute); dA = exp(delta*A[:,n]) on ScalarE;
    dBx = u*Bb, the two scans, and hc = hst*Cb ALL on DVE — the dBx/hc
    multiplies as single both-halves [128,2,2048] ops with a stride-0
    broadcast middle dim, which pipeline into DVE's stream nearly free,
    while cross-engine handoffs (GpSimd/split variants) cost 1.5-2.4x
    per-iteration (HW-measured via micro.py loop_* variants);
    y accumulates over n as identity matmuls in PSUM (8 banks).
  - yd = xc*D + y_psum; gate on GpSimd; out_proj lands in (t, d_model).
"""
import os
import numpy as np

D_MODEL, D_INNER, D_STATE, D_CONV, DT_RANK = 128, 256, 16, 4, 8
L = 2048
N_CORES = 8
NT = L // 128          # 16 t-tiles of 128
NC4 = L // 512         # 4 t-chunks of 512

_cache = {}


def _build(reps=1, legalize=True, hc_eng="vector", dbx_eng="vector", skip=(),
           silu_mode="act", crossrep=False, micro_opt=False):
    import concourse.bass as bass
    import concourse.tile as tile
    from concourse import mybir
    from concourse import masks

    f32 = mybir.dt.float32
    f16 = mybir.dt.float16
    ts = bass.ts
    Alu = mybir.AluOpType
    Act = mybir.ActivationFunctionType

    nc = bass.Bass()

    # ---- DRAM I/O (per core) ----
    x_d = nc.dram_tensor("x", [L, D_MODEL], f32, kind="ExternalInput")
    w1conv_d = nc.dram_tensor("w1conv", [128, 2, D_CONV, 128], f16, kind="ExternalInput")
    w1z_d = nc.dram_tensor("w1z", [128, 2, 128], f16, kind="ExternalInput")
    biasc_d = nc.dram_tensor("biasc", [128, 2], f32, kind="ExternalInput")
    biasz_d = nc.dram_tensor("biasz", [128, 2], f32, kind="ExternalInput")
    corr3_d = nc.dram_tensor("corr3", [128, 2, 3], f32, kind="ExternalInput")
    xpt_d = nc.dram_tensor("xpt", [128, 2, 96], f16, kind="ExternalInput")
    dtpt_d = nc.dram_tensor("dtpt", [DT_RANK, D_INNER], f16, kind="ExternalInput")
    dtb_d = nc.dram_tensor("dtb", [128, 2], f32, kind="ExternalInput")
    A_d = nc.dram_tensor("A", [128, 2, D_STATE], f32, kind="ExternalInput")
    Dp_d = nc.dram_tensor("Dp", [128, 2], f32, kind="ExternalInput")
    w2t_d = nc.dram_tensor("w2t", [128, 2, D_MODEL], f16, kind="ExternalInput")
    out_d = nc.dram_tensor("out", [L, D_MODEL], f32, kind="ExternalOutput")
    bc_d = nc.dram_tensor("bc_scratch", [2, D_STATE, L], f16, kind="Internal")

    eng = {"vector": nc.vector, "gpsimd": nc.gpsimd, "scalar": nc.scalar}
    hce = eng.get(hc_eng, nc.vector)
    dbe = eng.get(dbx_eng, nc.vector)

    with tile.TileContext(nc) as tc:
        with (
            tc.tile_pool(name="singles", bufs=1) as singles,
            tc.tile_pool(name="big", bufs=1) as big,
            tc.tile_pool(name="ln", bufs=4) as lnp,
            tc.tile_pool(name="scan", bufs=(2 if crossrep else 3)) as scanp,
            tc.tile_pool(name="bcast", bufs=(4 if micro_opt else 3)) as bcastp,
            tc.tile_pool(name="pp", bufs=1, space="PSUM") as pp,
        ):
            # ---- load weights ----
            w1conv = singles.tile([128, 2, D_CONV, 128], f16)
            nc.sync.dma_start(w1conv, w1conv_d[:])
            w1z = singles.tile([128, 2, 128], f16)
            nc.sync.dma_start(w1z, w1z_d[:])
            biasc = singles.tile([128, 2], f32)
            nc.sync.dma_start(biasc, biasc_d[:])
            biasz = singles.tile([128, 2], f32)
            nc.sync.dma_start(biasz, biasz_d[:])
            corr3 = singles.tile([128, 2, 3], f32)
            nc.sync.dma_start(corr3, corr3_d[:])
            xpt = singles.tile([128, 2, 96], f16)
            nc.sync.dma_start(xpt, xpt_d[:])
            dtpt = singles.tile([DT_RANK, D_INNER], f16)
            nc.sync.dma_start(dtpt, dtpt_d[:])
            dtb = singles.tile([128, 2], f32)
            nc.sync.dma_start(dtb, dtb_d[:])
            A_sb = singles.tile([128, 2, D_STATE], f32)
            nc.sync.dma_start(A_sb, A_d[:])
            Dp = singles.tile([128, 2], f32)
            nc.sync.dma_start(Dp, Dp_d[:])
            w2t = singles.tile([128, 2, D_MODEL], f16)
            nc.sync.dma_start(w2t, w2t_d[:])
            ident = singles.tile([128, 128], f16)
            masks.make_identity(nc, ident[:])
            eps = singles.tile([128, 1], f32)
            nc.vector.memset(eps, 1e-5)
            if silu_mode == "tanh":
                # silu(x) = x*0.5*(1+tanh(x/2)): halved biases for the
                # scale=0.5 activation reads
                biasc_h = singles.tile([128, 2], f32)
                nc.scalar.mul(biasc_h, biasc, 0.5)
                biasz_h = singles.tile([128, 2], f32)
                nc.scalar.mul(biasz_h, biasz, 0.5)

            bk = [f"bk{i}" for i in range(8)]
            xb = 2 if crossrep else 1

            for _rep in range(reps):
                # ---- load x: (2048, 128) -> (128 part, 16, 128) ----
                x_sb = big.tile([128, NT, D_MODEL], f32,
                                tag=("xin" if crossrep else "xio"), bufs=xb)
                nc.sync.dma_start(x_sb, x_d.rearrange("(i p) d -> p i d", p=128))

                # ---- LayerNorm (stats per t-row; t on partitions) ----
                # batched: all 16 tiles' stats first (DVE pipelines), then the
                # mean/var -> rstd/nmr math as a few [128,16]-wide ops
                xn16 = big.tile([128, NT, D_MODEL], f16)
                mv = lnp.tile([128, NT, 2], f32, tag="mv")
                for i in range(NT):
                    st = lnp.tile([128, 6], f32, tag=f"st{i % 4}")
                    nc.vector.bn_stats(st, x_sb[:, i, :])
                    nc.vector.bn_aggr(mv[:, i, :], st)
                # rstd = exp(-0.5*ln(var+eps)) — stays in the exp/ln
                # activation-table set (no sqrt table load)
                lv = lnp.tile([128, NT], f32, tag="lv")
                nc.scalar.activation(lv, mv[:, :, 1], Act.Ln, bias=eps[:])
                rstd = lnp.tile([128, NT], f32, tag="rstd")
                nc.scalar.activation(rstd, lv, Act.Exp, scale=-0.5)
                nmr = lnp.tile([128, NT], f32, tag="nmr")
                # nmr = -(mu * rstd)
                nc.vector.tensor_tensor(nmr, mv[:, :, 0], rstd, op=Alu.mult)
                nc.vector.tensor_scalar(nmr, nmr, -1.0, 0.0,
                                        op0=Alu.mult, op1=Alu.add)
                for i in range(NT):
                    nc.scalar.activation(xn16[:, i, :], x_sb[:, i, :], Act.Identity,
                                         bias=nmr[:, i:i + 1], scale=rstd[:, i:i + 1])

                # ---- transpose xn -> (d_model, 3+t) with 3 left pad cols ----
                xnT = big.tile([128, 3 + L], f16)
                nc.vector.memset(xnT[:, 0:3], 0.0)
                if micro_opt:
                    # 4 transposes per PSUM bank, one evac per group of 4
                    for g in range(4):
                        pt4 = pp.tile([128, 4, 128], f16, tag=bk[g % 4])
                        for j in range(4):
                            nc.tensor.transpose(pt4[:, j, :], xn16[:, 4 * g + j, :],
                                                ident)
                        if g % 2 == 0:
                            nc.vector.tensor_copy(
                                xnT[:, 3 + g * 512:3 + (g + 1) * 512],
                                pt4.rearrange("p a b -> p (a b)"))
                        else:
                            nc.scalar.copy(
                                xnT[:, 3 + g * 512:3 + (g + 1) * 512],
                                pt4.rearrange("p a b -> p (a b)"))
                else:
                    for i in range(NT):
                        pt = pp.tile([128, 128], f16, tag=bk[i % 6])
                        nc.tensor.transpose(pt, xn16[:, i, :], ident)
                        if i % 2 == 0:
                            nc.vector.tensor_copy(xnT[:, 3 + i * 128:3 + (i + 1) * 128], pt)
                        else:
                            nc.scalar.copy(xnT[:, 3 + i * 128:3 + (i + 1) * 128], pt)

                # ---- in_proj + fused causal conv (xc half), silu on evac ----
                xc2 = big.tile([128, 2, L], f16, name="xc2")
                sz2 = big.tile([128, 2, L], f16, name="sz2")

                def silu_evac(dst, pz, bias, bias_h, hh):
                    if silu_mode == "tanh":
                        # stays in the exp/ln/tanh act-table set (no
                        # swish-set reload): x*0.5*(1+tanh(x/2))
                        w = lnp.tile([128, 512], f16, tag="siw", bufs=3)
                        nc.scalar.activation(w, pz, Act.Identity, scale=0.5,
                                             bias=bias_h[:, hh:hh + 1])
                        t = lnp.tile([128, 512], f16, tag="sit", bufs=3)
                        nc.scalar.activation(t, pz, Act.Tanh, scale=0.5,
                                             bias=bias_h[:, hh:hh + 1])
                        nc.gpsimd.scalar_tensor_tensor(dst, t, 1.0, w,
                                                       op0=Alu.add, op1=Alu.mult)
                    else:
                        nc.scalar.activation(dst, pz, Act.Silu,
                                             bias=bias[:, hh:hh + 1])

                for h in range(2):
                    for tn in range(NC4):
                        pz = pp.tile([128, 512], f32, tag=bk[(h * 4 + tn) % 8])
                        for k in range(D_CONV):
                            nc.tensor.matmul(pz, w1conv[:, h, k, :],
                                             xnT[:, tn * 512 + k:tn * 512 + k + 512],
                                             start=(k == 0), stop=(k == D_CONV - 1))
                        if tn == 0:
                            # left-pad boundary: remove phantom-bias taps
                            nc.vector.tensor_tensor(pz[:, 0:3], pz[:, 0:3],
                                                    corr3[:, h, :], op=Alu.subtract)
                        silu_evac(xc2[:, h, ts(tn, 512)], pz, biasc,
                                  biasc_h if silu_mode == "tanh" else biasc, h)
                # ---- in_proj z half: sz = silu(z) on evac ----
                for g in range(2):
                    for tn in range(NC4):
                        pz = pp.tile([128, 512], f32, tag=bk[(g * 4 + tn) % 8])
                        nc.tensor.matmul(pz, w1z[:, g, :],
                                         xnT[:, 3 + tn * 512:3 + (tn + 1) * 512],
                                         start=True, stop=True)
                        silu_evac(sz2[:, g, ts(tn, 512)], pz, biasz,
                                  biasz_h if silu_mode == "tanh" else biasz, g)

                # ---- x_proj: dbc = x_proj_w @ xc  (96 rows: dt/B/C) ----
                dbc_sb = big.tile([96, L], f16, name="dbc")
                for tn in range(NC4):
                    pd = pp.tile([96, 512], f32, tag=bk[tn % 8])
                    nc.tensor.matmul(pd, xpt[:, 0, :], xc2[:, 0, ts(tn, 512)],
                                     start=True, stop=False)
                    nc.tensor.matmul(pd, xpt[:, 1, :], xc2[:, 1, ts(tn, 512)],
                                     start=False, stop=True)
                    nc.vector.tensor_copy(dbc_sb[:, ts(tn, 512)], pd)
                nc.sync.dma_start(bc_d[0], dbc_sb[32:32 + D_STATE, :])
                nc.sync.dma_start(bc_d[1], dbc_sb[64:64 + D_STATE, :])

                # ---- delta = softplus(dt_proj_w @ dt + b)  (d on partitions) ----
                delta2 = big.tile([128, 2, L], f16, name="delta2", bufs=xb)
                for h in range(2):
                    for tn in range(NC4):
                        pdl = pp.tile([128, 512], f32, tag=bk[4 + (h * 4 + tn) % 4])
                        nc.tensor.matmul(pdl, dtpt[:, ts(h, 128)],
                                         dbc_sb[0:DT_RANK, ts(tn, 512)],
                                         start=True, stop=True)
                        edl = lnp.tile([128, 512], f32, tag="edl", bufs=2)
                        nc.scalar.activation(edl, pdl, Act.Exp, bias=dtb[:, h:h + 1])
                        nc.scalar.activation(delta2[:, h, ts(tn, 512)], edl,
                                             Act.Ln, bias=1.0)

                # ---- u = delta * xc ----
                u2 = big.tile([128, 2, L], f16, name="u2", bufs=xb)
                nc.vector.tensor_tensor(u2, delta2, xc2, op=Alu.mult)

                # ---- selective scan, n-outer; y accumulates in PSUM ----
                py = [pp.tile([128, 512], f32, tag=bk[j], name=f"py{j}")
                      for j in range(8)]
                if "scanloop" in skip:
                    for j in range(8):
                        nc.tensor.matmul(py[j], ident, u2[:, j // 4, ts(j % 4, 512)],
                                         start=True, stop=True)
                for n in range(D_STATE if "scanloop" not in skip else 0):
                    if "bcast" in skip:
                        Bb = delta2[:, 0, :]
                        Cb = delta2[:, 1, :]
                    else:
                        # both on the SP HWDGE queue — issuing Cb from the
                        # scalar engine serializes against the dA exps
                        Bb = bcastp.tile([128, L], f16, tag="Bb")
                        nc.sync.dma_start(Bb, bc_d[0, n:n + 1, :].broadcast_to([128, L]))
                        Cb = bcastp.tile([128, L], f16, tag="Cb")
                        nc.sync.dma_start(Cb, bc_d[1, n:n + 1, :].broadcast_to([128, L]))

                    if "dA" in skip:
                        dA2 = u2
                    else:
                        dA2 = scanp.tile([128, 2, L], f16, tag="dA2")
                        for h in range(2):
                            nc.scalar.activation(dA2[:, h, :], delta2[:, h, :], Act.Exp,
                                                 scale=A_sb[:, h, n:n + 1])
                    if "dbx" in skip:
                        dBx2 = u2
                    else:
                        dBx2 = scanp.tile([128, 2, L], f16, tag="dBx2")
                        if dbx_eng == "split":
                            for h in range(2):
                                e = nc.vector if h == 0 else nc.gpsimd
                                e.tensor_tensor(dBx2[:, h, :], u2[:, h, :], Bb,
                                                op=Alu.mult)
                        elif dbx_eng in ("vector2", "gpsimd2"):
                            e = nc.vector if dbx_eng == "vector2" else nc.gpsimd
                            for h in range(2):
                                e.tensor_tensor(dBx2[:, h, :], u2[:, h, :], Bb,
                                                op=Alu.mult)
                        else:
                            dbe.tensor_tensor(dBx2, u2,
                                              Bb.unsqueeze(1).broadcast_to([128, 2, L]),
                                              op=Alu.mult)
                    if "scan" in skip:
                        hst2 = dBx2
                    else:
                        hst2 = scanp.tile([128, 2, L], f16, tag="hst2")
                        for h in range(2):
                            nc.vector.tensor_tensor_scan(hst2[:, h, :], dA2[:, h, :],
                                                         dBx2[:, h, :], 0.0,
                                                         op0=Alu.mult, op1=Alu.add)
                    if "hc" in skip:
                        hc2 = hst2
                    else:
                        hc2 = scanp.tile([128, 2, L], f16, tag="hc2")
                        if hc_eng == "split":
                            for h in range(2):
                                e = nc.gpsimd if h == 0 else nc.vector
                                e.tensor_tensor(hc2[:, h, :], hst2[:, h, :], Cb,
                                                op=Alu.mult)
                        elif hc_eng in ("vector2", "gpsimd2"):
                            e = nc.vector if hc_eng == "vector2" else nc.gpsimd
                            for h in range(2):
                                e.tensor_tensor(hc2[:, h, :], hst2[:, h, :], Cb,
                                                op=Alu.mult)
                        else:
                            hce.tensor_tensor(hc2, hst2,
                                              Cb.unsqueeze(1).broadcast_to([128, 2, L]),
                                              op=Alu.mult)
                    if "accum" in skip:
                        if n == 0:
                            for j in range(8):
                                nc.tensor.matmul(py[j], ident,
                                                 hc2[:, j // 4, ts(j % 4, 512)],
                                                 start=True, stop=True)
                    else:
                        for h in range(2):
                            for c in range(NC4):
                                nc.tensor.matmul(py[h * 4 + c], ident,
                                                 hc2[:, h, ts(c, 512)],
                                                 start=(n == 0), stop=(n == D_STATE - 1))

                # ---- yd = (xc * D) + y_psum;  gate;  out_proj ----
                yd2 = big.tile([128, 2, L], f16, name="yd2")
                yg2 = big.tile([128, 2, L], f16, name="yg2")
                out_sb = big.tile([128, NT, D_MODEL], f32, tag="xio")
                if micro_opt:
                    # chunked: gate + out_proj start as soon as each 512-wide
                    # chunk's yd is ready instead of after the full phase
                    for c in range(NC4):
                        for h in range(2):
                            nc.vector.scalar_tensor_tensor(
                                yd2[:, h, ts(c, 512)], xc2[:, h, ts(c, 512)],
                                Dp[:, h:h + 1], py[h * 4 + c],
                                op0=Alu.mult, op1=Alu.add)
                        nc.vector.tensor_tensor(yg2[:, :, ts(c, 512)],
                                                yd2[:, :, ts(c, 512)],
                                                sz2[:, :, ts(c, 512)], op=Alu.mult)
                        for i in range(4 * c, 4 * c + 4):
                            # only banks c and 4+c are free (their py was just
                            # consumed); later py banks are still live
                            po = pp.tile([128, D_MODEL], f32,
                                         tag=bk[c if i % 2 == 0 else 4 + c])
                            nc.tensor.matmul(po, yg2[:, 0, ts(i, 128)], w2t[:, 0, :],
                                             start=True, stop=False)
                            nc.tensor.matmul(po, yg2[:, 1, ts(i, 128)], w2t[:, 1, :],
                                             start=False, stop=True)
                            if i % 2 == 0:
                                nc.vector.tensor_copy(out_sb[:, i, :], po)
                            else:
                                nc.scalar.copy(out_sb[:, i, :], po)
                else:
                    for h in range(2):
                        for c in range(NC4):
                            nc.vector.scalar_tensor_tensor(
                                yd2[:, h, ts(c, 512)], xc2[:, h, ts(c, 512)],
                                Dp[:, h:h + 1], py[h * 4 + c], op0=Alu.mult, op1=Alu.add)
                    nc.vector.tensor_tensor(yg2, yd2, sz2, op=Alu.mult)

                    for i in range(NT):
                        po = pp.tile([128, D_MODEL], f32, tag=bk[i % 4])
                        nc.tensor.matmul(po, yg2[:, 0, ts(i, 128)], w2t[:, 0, :],
                                         start=True, stop=False)
                        nc.tensor.matmul(po, yg2[:, 1, ts(i, 128)], w2t[:, 1, :],
                                         start=False, stop=True)
                        if i % 2 == 0:
                            nc.vector.tensor_copy(out_sb[:, i, :], po)
                        else:
                            nc.scalar.copy(out_sb[:, i, :], po)

                nc.sync.dma_start(out_d.rearrange("(i p) d -> p i d", p=128), out_sb)

    if legalize:
        _legalize_waits(nc)
    return nc


def _legalize_waits(nc):
    """This container's walrus codegen rejects instructions carrying more
    than one sync wait. Hoist extra waits onto preceding wait-only
    InstEventSemaphore instructions on the same engine (sequencers execute
    them in order, so the semantics are identical)."""
    from concourse import mybir

    fixid = [0]
    for fn in nc.m.functions:
        for blk in fn.blocks:
            out = []
            changed = False
            for ins in blk.instructions:
                si = getattr(ins, "sync_info", None)
                waits = list(si.on_wait) if si is not None and si.on_wait else []
                if len(waits) > 1:
                    for w in waits[:-1]:
                        fixid[0] += 1
                        nop = mybir.InstEventSemaphore(
                            name=f"I-waitfix-{fixid[0]}", ins=[], outs=[],
                            sync_info=mybir.SyncInfo(on_wait=[w], on_update=[]))
                        nop.engine = ins.engine
                        out.append(nop)
                    ins.sync_info = mybir.SyncInfo(
                        on_wait=[waits[-1]], on_update=list(si.on_update))
                    changed = True
                out.append(ins)
            if changed:
                blk.instructions = out


def _prep_inputs(x, norm_w, norm_b, in_proj_w, conv_w, conv_b, x_proj_w,
                 dt_proj_w, dt_proj_b, A_log, D, out_proj_w):
    """Host-side weight prep; returns per-core input maps."""
    f32 = np.float32
    f16 = np.float16
    W1eff = (in_proj_w.astype(f32) * norm_w.astype(f32)[None, :])  # (512, 128)
    b1 = (in_proj_w.astype(f32) @ norm_b.astype(f32))              # (512,)
    cw = conv_w.astype(f32)                                        # (256, 4)

    # xc half: 4 tap-scaled stationaries  w1conv[d, h, k, j] = W1eff[h*128+j, d]*cw[h*128+j, k]
    Wt = W1eff[:256].T                                             # (128d, 256c)
    w1conv = np.empty((128, 2, D_CONV, 128), f32)
    for h in range(2):
        for k in range(D_CONV):
            w1conv[:, h, k, :] = Wt[:, h * 128:(h + 1) * 128] * cw[h * 128:(h + 1) * 128, k][None, :]
    w1conv = w1conv.astype(f16)
    w1z = np.ascontiguousarray(
        W1eff[256:].T.reshape(128, 2, 128)).astype(f16)
    # conv bias folded with in_proj bias (all 4 taps of b1)
    biasc = (conv_b.astype(f32) + b1[:256] * cw.sum(1)).reshape(2, 128).T
    biasc = np.ascontiguousarray(biasc)
    biasz = np.ascontiguousarray(b1[256:].reshape(2, 128).T)
    # boundary correction: output col j<3 should not include b1 for taps k<3-j
    corr3 = np.empty((128, 2, 3), f32)
    for h in range(2):
        for j in range(3):
            corr3[:, h, j] = b1[h * 128:(h + 1) * 128] * cw[h * 128:(h + 1) * 128, :3 - j].sum(1)
    corr3 = np.ascontiguousarray(corr3)

    xpw_pad = np.zeros((96, 256), f32)
    xpw_pad[0:8] = x_proj_w[0:8]
    xpw_pad[32:48] = x_proj_w[8:24]
    xpw_pad[64:80] = x_proj_w[24:40]
    xpt = np.ascontiguousarray(
        xpw_pad.T.reshape(2, 128, 96).transpose(1, 0, 2)).astype(f16)
    dtpt = np.ascontiguousarray(dt_proj_w.astype(f32).T).astype(f16)  # (8, 256)
    dtb = np.ascontiguousarray(dt_proj_b.astype(f32).reshape(2, 128).T)
    A = (-np.exp(A_log.astype(f32)))
    A = np.ascontiguousarray(A.reshape(2, 128, D_STATE).transpose(1, 0, 2))
    Dp = np.ascontiguousarray(D.astype(f32).reshape(2, 128).T)
    w2t = np.ascontiguousarray(
        out_proj_w.astype(f32).T.reshape(2, 128, D_MODEL).transpose(1, 0, 2)).astype(f16)

    shared = dict(w1conv=w1conv, w1z=w1z, biasc=biasc, biasz=biasz, corr3=corr3,
                  xpt=xpt, dtpt=dtpt, dtb=dtb, A=A, Dp=Dp, w2t=w2t)
    in_maps = []
    for b in range(N_CORES):
        m = dict(shared)
        m["x"] = np.ascontiguousarray(x[b].astype(f32))
        in_maps.append(m)
    return in_maps


def kernel(**inputs):
    from concourse.bass_utils import run_bass_kernel_spmd

    if "nc" not in _cache:
        _cache["nc"] = _build(micro_opt=True)
    nc = _cache["nc"]

    x = np.asarray(inputs["x"])
    in_maps = _prep_inputs(**{k: np.asarray(v) for k, v in inputs.items()})
    res = run_bass_kernel_spmd(nc, in_maps, list(range(N_CORES)),
                               trace=bool(int(os.environ.get("KTRACE", "0"))))
    _cache["last_results"] = res
    out = np.stack([res.results[b]["out"] for b in range(N_CORES)]).astype(np.float32)
    residual = x.astype(np.float32).copy()
    return out, residual
